# revision 1
# baseline (speedup 1.0000x reference)
"""Corner-pooling module kernel for 8 Trainium2 NeuronCores.

Reference computation (NCHW, fp32):
    p1 = relu(bn(conv3x3(x, w_p1)))          # 256 -> 128 ch
    p2 = relu(bn(conv3x3(x, w_p2)))          # 256 -> 128 ch
    cp1 = cummax(p1, axis=H, reverse=True)
    cp2 = cummax(p2, axis=W, reverse=True)
    r  = relu(bn(conv3x3(cp1+cp2, w_c1)) + bn(conv1x1(x, w_c2)))
    out = relu(bn(conv3x3(r, w_p3)))

Sharding: 8 cores = 4 samples x 2 H-halves (core 2b: rows 0..63 of
sample b, core 2b+1: rows 64..127).  All BN scales are folded into the
conv weights host-side (per-out-channel bias remains), so every conv is
matmul-accumulate + one ACT relu-with-bias epilogue.  Convs run as
flat-chunked matmuls over a W-padded (130-wide) layout with channels on
partitions, in float32r (full-rate PE, ~2.4e-4 eps).  The reverse
cummaxes are a 63-step DVE max chain (H) and a reversed
tensor_tensor_scan with -3e38 row resets (W).  The single cross-core
dependency (column max of the bottom half for the top core's cummax,
plus 3x3 conv halo rows) is one pairwise AllReduce of 8 rows, made
uniform across cores by 0/1 mask inputs.
"""

import sys

sys.path.insert(0, "/opt/trn_rl_repo")

import numpy as np

import bass_rust
import concourse.bass as bass
import concourse.mybir as mybir
from concourse.bass_utils import run_bass_kernel_spmd
from concourse.tile import TileContext

F32 = mybir.dt.float32
F32R = mybir.dt.float32r
EPS = 1e-5

B, CIN, H, W = 4, 256, 128, 128
MID = 128
P = 128
WP = W + 2          # padded width
HS = 66             # x-slab rows: own 64 + 1 halo above + 1 below
SS = 68             # s-slab rows: own 64 + 2 halo above + 2 below
OH = 64             # own rows per core
TAPS = [(ky, kx) for ky in (-1, 0, 1) for kx in (-1, 0, 1)]


def _r12(x):
    """Round fp32 to float32r precision (11 explicit mantissa bits, RNE)."""
    u = np.ascontiguousarray(x, dtype=np.float32).view(np.uint32)
    r = (u + 0x7FF + ((u >> 12) & 1)) & 0xFFFFF000
    return r.view(np.float32)


def _fix_multiwaits(nc):
    """walrus in this container accepts at most ONE sem wait per
    instruction; split extras onto same-engine nops placed just before."""

    def steal_nop(eng):
        bi = nc.engines[eng].nop()
        ins = bi.ins
        cur = nc.cur_bb.bb
        lst = cur.instructions
        assert lst[-1] is ins or lst[-1].name == ins.name
        cur.instructions = lst[:-1]
        return ins

    for fn in nc.m.functions:
        for bb in fn.blocks:
            out = []
            changed = False
            for inst in bb.instructions:
                si = inst.sync_info
                waits = list(si.on_wait) if si is not None and si.on_wait else []
                if len(waits) > 1:
                    for wv in waits[:-1]:
                        nop = steal_nop(inst.engine)
                        nop.sync_info = bass_rust.SyncInfo(on_wait=[wv], on_update=[])
                        out.append(nop)
                    inst.sync_info = bass_rust.SyncInfo(
                        on_wait=[waits[-1]], on_update=list(si.on_update or [])
                    )
                    changed = True
                out.append(inst)
            if changed:
                bb.instructions = out


def _emit_conv(nc, psum, rhs_flats, rhs_sizes, terms, nt, nchunks, epilogue):
    """Flat-chunked conv: for each output chunk accumulate all terms'
    matmuls in one PSUM tile, then run epilogue(chunk_idx, psum_ap).

    terms: list of (rhs_idx, lhsT_ap, off) where the term reads
    rhs_flats[rhs_idx][:, q+off : q+off+nt] for out positions [q, q+nt).
    Edge terms are trimmed by one element at the slab ends.
    """
    for ci in range(nchunks):
        q = ci * nt
        full, part = [], []
        for rhs_idx, lhsT, off in terms:
            o = q + off
            lo = max(0, -o)
            hi = min(nt, rhs_sizes[rhs_idx] - o)
            # fp32r matmul dst needs even offset+length; the extra trimmed
            # elements only ever read zero pad positions
            lo += lo & 1
            hi -= (hi - lo) & 1
            (full if (lo == 0 and hi == nt) else part).append(
                (rhs_idx, lhsT, o, lo, hi)
            )
        seq = [full[0]] + part + full[1:]
        pt = psum.tile([P, 512], F32, tag="ps")
        n = len(seq)
        for i, (rhs_idx, lhsT, o, lo, hi) in enumerate(seq):
            nc.tensor.matmul(
                pt[:, lo:hi],
                lhsT,
                rhs_flats[rhs_idx][:, o + lo : o + hi],
                start=(i == 0),
                stop=(i == n - 1),
            )
        epilogue(ci, pt[:, :nt])


def build_nc():
    nc = bass.Bass()

    xs = nc.dram_tensor("xs", [2, P, HS, WP], F32R, kind="ExternalInput")
    wp1 = nc.dram_tensor("wp1", [P, 2, 9, P], F32R, kind="ExternalInput")
    wp2 = nc.dram_tensor("wp2", [P, 2, 9, P], F32R, kind="ExternalInput")
    wc1 = nc.dram_tensor("wc1", [P, 2, 9, P], F32R, kind="ExternalInput")
    wc2 = nc.dram_tensor("wc2", [P, 2, 2, P], F32R, kind="ExternalInput")
    wp3 = nc.dram_tensor("wp3", [P, 2, 2, 9, P], F32R, kind="ExternalInput")
    bias = nc.dram_tensor("bias", [P, 6], F32, kind="ExternalInput")
    msk = nc.dram_tensor("msk", [P, 3], F32, kind="ExternalInput")
    o = nc.dram_tensor("o", [2, P, OH, W], F32, kind="ExternalOutput")

    groups = [[0, 1], [2, 3], [4, 5], [6, 7]]

    with TileContext(nc) as tc:
        with (
            tc.tile_pool(name="const", bufs=1) as cpool,
            tc.tile_pool(name="wt", bufs=2) as wpool,
            tc.tile_pool(name="psum", bufs=6, space="PSUM") as psum,
            tc.tile_pool(name="scratch", bufs=1) as spool_s,
            tc.tile_pool(name="ost", bufs=3) as opool,
            tc.tile_pool(name="dram", bufs=1, space="DRAM") as dpool,
            tc.tile_pool(name="sbig", bufs=1) as spool,
            tc.tile_pool(name="x", bufs=1) as xpool,
        ):
            # ---- constants ----
            bt = cpool.tile([P, 6], F32)
            nc.sync.dma_start(bt[:], bias[:])
            mt_ = cpool.tile([P, 3], F32)
            nc.sync.dma_start(mt_[:], msk[:])
            m_top = mt_[:, 0:1]
            m_bot = mt_[:, 1:2]
            coloff = mt_[:, 2:3]
            wc2t = cpool.tile([P, 2, 2, P], F32R)
            nc.sync.dma_start(wc2t[:], wc2[:])

            # ---- x slabs ----
            xt = [xpool.tile([P, HS, WP], F32R, name=f"x{s}", tag=f"x{s}") for s in range(2)]
            for s in range(2):
                for r0 in range(0, HS, 6):
                    r1 = min(r0 + 6, HS)
                    nc.sync.dma_start(xt[s][:, r0:r1, :], xs[s, :, r0:r1, :])
            xf = [t[:].rearrange("p h w -> p (h w)") for t in xt]
            xsz = HS * WP

            # ---- weights (phased through 2 slots) ----
            def wtile(src):
                t = wpool.tile([P, 2, 9, P], F32R, tag="w")
                nc.sync.dma_start(t[:], src[:])
                return t

            wp2t = wtile(wp2)
            wp1t = wtile(wp1)

            # ---- s slab (conv p1 writes rows 2..65; becomes cp1, then s) ----
            st = spool.tile([P, SS, WP], F32R)
            sf = st[:].rearrange("p h w -> p (h w)")
            s_own = st[:, 2 : 2 + OH, :]
            s_own_f = s_own.rearrange("p h w -> p (h w)")

            NT_A, NCH_A = 416, (OH * WP) // 416

            with tc.tile_pool(name="p2", bufs=1) as p2pool:
                p2t = p2pool.tile([P, OH, WP], F32R)
                p2f = p2t[:].rearrange("p h w -> p (h w)")

                def conv_branch(wt_tile, bias_col, out_flat, out_base):
                    terms = []
                    for s in range(2):
                        for t_i, (ky, kx) in enumerate(TAPS):
                            terms.append(
                                (s, wt_tile[:, s, t_i, :], (1 + ky) * WP + kx)
                            )

                    def epi(ci, pap):
                        ncols = pap.shape[-1]
                        nc.scalar.activation(
                            out_flat[:, out_base + ci * NT_A : out_base + ci * NT_A + ncols],
                            pap,
                            mybir.ActivationFunctionType.Relu,
                            bias=bias_col,
                        )

                    _emit_conv(nc, psum, xf, [xsz, xsz], terms, NT_A, NCH_A, epi)

                conv_branch(wp2t, bt[:, 1:2], p2f, 0)
                # reverse cummax over W: one reversed scan per row over the
                # real cols 1..128 (state starts at -1e30 per row)
                for h in range(OH):
                    row = p2t[:, h, 1 : WP - 1]
                    nc.vector.tensor_tensor_scan(
                        row[:, ::-1],
                        row[:, ::-1],
                        row[:, ::-1],
                        -1e30,
                        mybir.AluOpType.max,
                        mybir.AluOpType.bypass,
                    )

                # conv p1 -> s rows 2..65
                conv_branch(wp1t, bt[:, 0:1], sf, 2 * WP)
                # reverse cummax over H (rows 64 -> 2), in place
                for h in range(OH, 1, -1):
                    nc.vector.tensor_tensor(
                        st[:, h, :],
                        st[:, h, :],
                        st[:, h + 1, :],
                        mybir.AluOpType.max,
                    )

                # ---- pairwise exchange ----
                # C slots: 0,1 cp1local first2 rows (*mb); 2,3 cp1local last2
                # (*mt); 4,5 cp2 first2 (*mb); 6,7 cp2 last2 (*mt)
                ct = spool_s.tile([P, 8, WP], F32, tag="exch")
                for k, (src, m) in enumerate(
                    [
                        (st[:, 2, :], m_bot),
                        (st[:, 3, :], m_bot),
                        (st[:, 2 + OH - 2, :], m_top),
                        (st[:, 2 + OH - 1, :], m_top),
                        (p2t[:, 0, :], m_bot),
                        (p2t[:, 1, :], m_bot),
                        (p2t[:, OH - 2, :], m_top),
                        (p2t[:, OH - 1, :], m_top),
                    ]
                ):
                    nc.vector.tensor_scalar_mul(ct[:, k, :], src, m)
                cc_in = dpool.tile([P, 8, WP], F32)
                cc_out = dpool.tile([P, 8, WP], F32)
                nc.sync.dma_start(cc_in[:], ct[:])
                nc.gpsimd.collective_compute(
                    "AllReduce",
                    mybir.AluOpType.add,
                    replica_groups=groups,
                    ins=[cc_in[:]],
                    outs=[cc_out[:]],
                )
                rt = spool_s.tile([P, 8, WP], F32R, tag="exch")
                nc.sync.dma_start(rt[:], cc_out[:].bitcast(F32R))

                # colmax fixup: u = R[0] + coloff ; cp1 = max(cp1, u)
                u = spool_s.tile([P, WP], F32R, tag="u")
                nc.vector.tensor_scalar_add(u[:], rt[:, 0, :], coloff)
                nc.vector.tensor_tensor(
                    s_own,
                    s_own,
                    u[:, None, :].to_broadcast((P, OH, WP)),
                    mybir.AluOpType.max,
                )
                # save own colmax (= first own row of fixed cp1)
                cm = spool_s.tile([P, WP], F32R, tag="cm")
                nc.vector.tensor_copy(cm[:], st[:, 2, :])
                # s = cp1 + cp2
                nc.vector.tensor_tensor(
                    s_own_f, s_own_f, p2f[:], mybir.AluOpType.add
                )

            # halo rows of s (outside p2pool; p2 no longer needed)
            h0 = spool_s.tile([P, WP], F32R, tag="h0")
            h1 = spool_s.tile([P, WP], F32R, tag="h1")
            rt_ = rt  # keep name
            # above halo (bottom cores): max(partner cp1local last2, own
            # colmax) + partner cp2 last2, masked by mb
            for j, dst_row in ((0, 0), (1, 1)):
                nc.vector.tensor_tensor(
                    h0[:], rt_[:, 2 + j, :], cm[:], mybir.AluOpType.max
                )
                nc.vector.tensor_tensor(
                    h0[:], h0[:], rt_[:, 6 + j, :], mybir.AluOpType.add
                )
                nc.vector.tensor_scalar_mul(st[:, dst_row, :], h0[:], m_bot)
            # below halo (top cores): partner cp1local first2 + cp2 first2
            for j, dst_row in ((0, SS - 2), (1, SS - 1)):
                nc.vector.tensor_tensor(
                    h1[:], rt_[:, 0 + j, :], rt_[:, 4 + j, :], mybir.AluOpType.add
                )
                nc.vector.tensor_scalar_mul(st[:, dst_row, :], h1[:], m_top)
            # zero pad cols of s (in-place *0: memset may not produce
            # fp32r-matmul operands under this walrus; values are finite)
            nc.vector.tensor_scalar_mul(st[:, :, 0:1], st[:, :, 0:1], 0.0)
            nc.vector.tensor_scalar_mul(
                st[:, :, WP - 1 : WP], st[:, :, WP - 1 : WP], 0.0
            )

            # ---- conv c1 + c2 -> r_ext ----
            wc1t = wtile(wc1)
            NT_C, NCH_C = 390, ((OH + 2) * WP) // 390
            ssz = SS * WP
            with tc.tile_pool(name="r", bufs=1) as rpool:
                rt2 = [rpool.tile([P, HS, WP], F32R, name=f"r{i}", tag=f"r{i}") for i in range(2)]
                rf = [t[:].rearrange("p h w -> p (h w)") for t in rt2]
                for oh_half in range(2):
                    terms = []
                    for t_i, (ky, kx) in enumerate(TAPS):
                        terms.append((0, wc1t[:, oh_half, t_i, :], (1 + ky) * WP + kx))
                    for s in range(2):
                        terms.append((1 + s, wc2t[:, s, oh_half, :], 0))

                    def epi(ci, pap, oh_half=oh_half):
                        ncols = pap.shape[-1]
                        nc.scalar.activation(
                            rf[oh_half][:, ci * NT_C : ci * NT_C + ncols],
                            pap,
                            mybir.ActivationFunctionType.Relu,
                            bias=bt[:, 2 + oh_half : 3 + oh_half],
                        )

                    _emit_conv(
                        nc, psum, [sf, xf[0], xf[1]], [ssz, xsz, xsz],
                        terms, NT_C, NCH_C, epi,
                    )
                # mask invalid halo rows, zero pad cols
                for t in rt2:
                    nc.vector.tensor_scalar_mul(t[:, 0, :], t[:, 0, :], m_bot)
                    nc.vector.tensor_scalar_mul(
                        t[:, HS - 1, :], t[:, HS - 1, :], m_top
                    )
                    nc.vector.tensor_scalar_mul(t[:, :, 0:1], t[:, :, 0:1], 0.0)
                    nc.vector.tensor_scalar_mul(
                        t[:, :, WP - 1 : WP], t[:, :, WP - 1 : WP], 0.0
                    )

                # ---- conv p3 -> out ----
                wp3a = wpool.tile([P, 2, 9, P], F32R, tag="w")
                nc.sync.dma_start(wp3a[:], wp3[:, 0])
                wp3b = wpool.tile([P, 2, 9, P], F32R, tag="w")
                nc.sync.dma_start(wp3b[:], wp3[:, 1])
                NT_O, NCH_O = 260, (OH * WP) // 260
                rsz = HS * WP
                for oh_half, wtile_ in ((0, wp3a), (1, wp3b)):
                    terms = []
                    for s in range(2):
                        for t_i, (ky, kx) in enumerate(TAPS):
                            terms.append(
                                (s, wtile_[:, s, t_i, :], (1 + ky) * WP + kx)
                            )

                    def epi(ci, pap, oh_half=oh_half):
                        ncols = pap.shape[-1]
                        ot = opool.tile([P, 2, WP], F32, tag="ost")
                        nc.scalar.activation(
                            ot[:].rearrange("p r w -> p (r w)")[:, :ncols],
                            pap,
                            mybir.ActivationFunctionType.Relu,
                            bias=bt[:, 4 + oh_half : 5 + oh_half],
                        )
                        nc.sync.dma_start(
                            o[oh_half, :, ci * 2 : ci * 2 + 2, :],
                            ot[:, :, 1 : WP - 1],
                        )

                    _emit_conv(nc, psum, rf, [rsz, rsz], terms, NT_O, NCH_O, epi)

    _fix_multiwaits(nc)
    return nc


_NC = None


def _get_nc():
    global _NC
    if _NC is None:
        _NC = build_nc()
    return _NC


def _fold_bn(w, g, b, m, v):
    s = (g / np.sqrt(v + EPS)).astype(np.float32)
    t = (b - m * s).astype(np.float32)
    return w * s[:, None, None, None], t


def kernel(**inputs):
    x = np.asarray(inputs["x"], np.float32)

    w_p1, t_p1 = _fold_bn(
        np.asarray(inputs["w_p1"], np.float32),
        inputs["g_p1"], inputs["b_p1"], inputs["m_p1"], inputs["v_p1"],
    )
    w_p2, t_p2 = _fold_bn(
        np.asarray(inputs["w_p2"], np.float32),
        inputs["g_p2"], inputs["b_p2"], inputs["m_p2"], inputs["v_p2"],
    )
    w_c1, t_c1 = _fold_bn(
        np.asarray(inputs["w_c1"], np.float32),
        inputs["g_c1"], inputs["b_c1"], inputs["m_c1"], inputs["v_c1"],
    )
    w_c2, t_c2 = _fold_bn(
        np.asarray(inputs["w_c2"], np.float32),
        inputs["g_c2"], inputs["b_c2"], inputs["m_c2"], inputs["v_c2"],
    )
    w_p3, t_p3 = _fold_bn(
        np.asarray(inputs["w_p3"], np.float32),
        inputs["g_p3"], inputs["b_p3"], inputs["m_p3"], inputs["v_p3"],
    )

    # weight layouts (see build_nc): contraction channel on partitions
    def lay3x3(wf, cout_half):
        # wf [O, I, 3, 3] -> [128ci, n_i_sub, (oh?), 9, 128co]
        O, I = wf.shape[0], wf.shape[1]
        a = wf.reshape(O // P, P, I // P, P, 3, 3)  # [ohs, co, s, ci, ky, kx]
        a = a.transpose(3, 0, 2, 4, 5, 1)  # [ci, ohs, s, ky, kx, co]
        return np.ascontiguousarray(a)

    wp1a = lay3x3(w_p1, 1).reshape(P, 1, 2, 9, P)[:, 0]          # [128,2,9,128]
    wp2a = lay3x3(w_p2, 1).reshape(P, 1, 2, 9, P)[:, 0]
    wc1a = lay3x3(w_c1, 2).reshape(P, 2, 1, 9, P)[:, :, 0]       # [128,2oh,9,128]
    wp3a = lay3x3(w_p3, 2)                                        # [128,2oh,2s,9,128]
    wc2a = np.ascontiguousarray(
        w_c2[:, :, 0, 0].reshape(2, P, 2, P).transpose(3, 2, 0, 1)
    )  # [128ci, 2s, 2oh, 128co]

    bias = np.zeros((P, 6), np.float32)
    bias[:, 0] = t_p1
    bias[:, 1] = t_p2
    bc = t_c1 + t_c2
    bias[:, 2] = bc[:P]
    bias[:, 3] = bc[P:]
    bias[:, 4] = t_p3[:P]
    bias[:, 5] = t_p3[P:]

    # x slabs per core, W-padded + H halo, rounded to f32r
    xr = x.reshape(B, 2, P, H, W)
    slabs = np.zeros((B, 2, 2, P, HS, WP), np.float32)  # [b, half, s, p, h, w]
    for b in range(B):
        slabs[b, 0, :, :, 1:HS, 1 : WP - 1] = xr[b, :, :, 0:65, :]
        slabs[b, 1, :, :, 0 : HS - 1, 1 : WP - 1] = xr[b, :, :, 63:128, :]
    slabs = _r12(slabs)

    wmaps = {
        "wp1": _r12(wp1a),
        "wp2": _r12(wp2a),
        "wc1": _r12(wc1a),
        "wc2": _r12(wc2a),
        "wp3": _r12(wp3a),
        "bias": bias,
    }
    in_maps = []
    for b in range(B):
        for half in range(2):
            msk = np.zeros((P, 3), np.float32)
            if half == 0:  # top
                msk[:, 0] = 1.0  # m_top
                msk[:, 2] = 0.0
            else:  # bottom
                msk[:, 1] = 1.0  # m_bot
                msk[:, 2] = -1e30
            in_maps.append({"xs": slabs[b, half], "msk": msk, **wmaps})

    global _last_in_maps
    _last_in_maps = in_maps

    nc = _get_nc()
    res = run_bass_kernel_spmd(nc, in_maps, list(range(8)))

    out = np.empty((B, CIN, H, W), np.float32)
    for b in range(B):
        out[b, :, 0:OH] = res.results[2 * b]["o"].reshape(CIN, OH, W)
        out[b, :, OH:H] = res.results[2 * b + 1]["o"].reshape(CIN, OH, W)
    return out


if __name__ == "__main__":
    import reference

    inp = {k: np.asarray(v) for k, v in reference.setup_inputs().items()}
    exp = np.asarray(reference.reference(**inp))
    got = kernel(**inp)
    err = np.abs(got - exp)
    rel = err.max() / max(np.abs(exp).max(), 1e-6)
    print("abs err max:", err.max(), "rel (vs absmax):", rel)



# revision 5
# speedup vs baseline: 1.1888x; 1.1888x over previous
"""Corner-pooling module kernel for 8 Trainium2 NeuronCores.

Reference computation (NCHW, fp32):
    p1 = relu(bn(conv3x3(x, w_p1)))          # 256 -> 128 ch
    p2 = relu(bn(conv3x3(x, w_p2)))          # 256 -> 128 ch
    cp1 = cummax(p1, axis=H, reverse=True)
    cp2 = cummax(p2, axis=W, reverse=True)
    r  = relu(bn(conv3x3(cp1+cp2, w_c1)) + bn(conv1x1(x, w_c2)))
    out = relu(bn(conv3x3(r, w_p3)))

Sharding: 8 cores = 4 samples x 2 H-halves (core 2b: rows 0..63 of
sample b, core 2b+1: rows 64..127).  All BN scales are folded into the
conv weights host-side (per-out-channel bias remains), so every conv is
matmul-accumulate + one ACT relu-with-bias epilogue.  Convs run as
flat-chunked matmuls over a W-padded (130-wide) layout with channels on
partitions, in float32r (full-rate PE, ~2.4e-4 eps).

Schedule (the point of this version): conv p1 runs FIRST in reverse
chunk order so the 63-step reverse H-cummax chain interleaves under it
on DVE; conv p2 runs second with its first+last chunks prioritized so
the 8-row pairwise AllReduce (the only cross-core dependency) plus all
cummax fixups overlap p2's remaining matmuls.  The W-cummax runs as 4
flat multi-row reversed scans using a (max,mult) mask-reset trick, and
the colmax-broadcast fixup / s=cp1+cp2 add / pad zeroing are split into
4 row pieces (fixups on DVE, adds on gpsimd) so conv c1 starts the
moment p2's matmuls end.  x streams highest-rows-first so conv p1's
first (reverse) chunk starts ~4us in, and a few dummy matmuls on the
wc2 tile pre-warm the PE clock gate during the initial DMA.
"""

import sys

sys.path.insert(0, "/opt/trn_rl_repo")

import numpy as np

import bass_rust
import concourse.bass as bass
import concourse.mybir as mybir
from concourse.bass_utils import run_bass_kernel_spmd
from concourse.tile import TileContext

F32 = mybir.dt.float32
F32R = mybir.dt.float32r
EPS = 1e-5

B, CIN, H, W = 4, 256, 128, 128
MID = 128
P = 128
WP = W + 2          # padded width
HS = 66             # x-slab rows: own 64 + 1 halo above + 1 below
SS = 68             # s-slab rows: own 64 + 2 halo above + 2 below
OH = 64             # own rows per core
TAPS = [(ky, kx) for ky in (-1, 0, 1) for kx in (-1, 0, 1)]
NPC = 4             # fixup/add/scan pieces
PCR = OH // NPC     # rows per piece


def _r12(x):
    """Round fp32 to float32r precision (11 explicit mantissa bits, RNE)."""
    u = np.ascontiguousarray(x, dtype=np.float32).view(np.uint32)
    r = (u + 0x7FF + ((u >> 12) & 1)) & 0xFFFFF000
    return r.view(np.float32)


def _fix_multiwaits(nc):
    """walrus in this container accepts at most ONE sem wait per
    instruction; split extras onto same-engine nops placed just before."""

    def steal_nop(eng):
        bi = nc.engines[eng].nop()
        ins = bi.ins
        cur = nc.cur_bb.bb
        lst = cur.instructions
        assert lst[-1] is ins or lst[-1].name == ins.name
        cur.instructions = lst[:-1]
        return ins

    for fn in nc.m.functions:
        for bb in fn.blocks:
            out = []
            changed = False
            for inst in bb.instructions:
                si = inst.sync_info
                waits = list(si.on_wait) if si is not None and si.on_wait else []
                if len(waits) > 1:
                    for wv in waits[:-1]:
                        nop = steal_nop(inst.engine)
                        nop.sync_info = bass_rust.SyncInfo(on_wait=[wv], on_update=[])
                        out.append(nop)
                    inst.sync_info = bass_rust.SyncInfo(
                        on_wait=[waits[-1]], on_update=list(si.on_update or [])
                    )
                    changed = True
                out.append(inst)
            if changed:
                bb.instructions = out
    return nc


def _emit_conv(nc, psum, rhs_flats, rhs_sizes, terms, nt, nchunks, epilogue,
               order=None):
    """Flat-chunked conv: for each output chunk accumulate all terms'
    matmuls in one PSUM tile, then run epilogue(chunk_idx, psum_ap).

    terms: list of (rhs_idx, lhsT_ap, off) where the term reads
    rhs_flats[rhs_idx][:, q+off : q+off+nt] for out positions [q, q+nt).
    Edge terms are trimmed by one element at the slab ends.
    order: chunk iteration order (default ascending).
    """
    for ci in (range(nchunks) if order is None else order):
        q = ci * nt
        full, part = [], []
        for rhs_idx, lhsT, off in terms:
            o = q + off
            lo = max(0, -o)
            hi = min(nt, rhs_sizes[rhs_idx] - o)
            # fp32r matmul dst needs even offset+length; the extra trimmed
            # elements only ever read zero pad positions
            lo += lo & 1
            hi -= (hi - lo) & 1
            (full if (lo == 0 and hi == nt) else part).append(
                (rhs_idx, lhsT, o, lo, hi)
            )
        seq = [full[0]] + part + full[1:]
        pt = psum.tile([P, 512], F32, tag="ps")
        n = len(seq)
        for i, (rhs_idx, lhsT, o, lo, hi) in enumerate(seq):
            nc.tensor.matmul(
                pt[:, lo:hi],
                lhsT,
                rhs_flats[rhs_idx][:, o + lo : o + hi],
                start=(i == 0),
                stop=(i == n - 1),
            )
        epilogue(ci, pt[:, :nt])


def build_nc():
    nc = bass.Bass()

    xs = nc.dram_tensor("xs", [2, P, HS, WP], F32R, kind="ExternalInput")
    wp1 = nc.dram_tensor("wp1", [P, 2, 9, P], F32R, kind="ExternalInput")
    wp2 = nc.dram_tensor("wp2", [P, 2, 9, P], F32R, kind="ExternalInput")
    wc1 = nc.dram_tensor("wc1", [P, 2, 9, P], F32R, kind="ExternalInput")
    wc2 = nc.dram_tensor("wc2", [P, 2, 2, P], F32R, kind="ExternalInput")
    wp3 = nc.dram_tensor("wp3", [P, 2, 2, 9, P], F32R, kind="ExternalInput")
    bias = nc.dram_tensor("bias", [P, 6], F32, kind="ExternalInput")
    msk = nc.dram_tensor("msk", [P, 3], F32, kind="ExternalInput")
    o = nc.dram_tensor("o", [2, P, OH, W], F32, kind="ExternalOutput")

    groups = [[0, 1], [2, 3], [4, 5], [6, 7]]

    with TileContext(nc) as tc:
        with (
            tc.tile_pool(name="const", bufs=1) as cpool,
            tc.tile_pool(name="wt", bufs=2) as wpool,
            tc.tile_pool(name="psum", bufs=6, space="PSUM") as psum,
            tc.tile_pool(name="scratch", bufs=1) as spool_s,
            tc.tile_pool(name="ost", bufs=3) as opool,
            tc.tile_pool(name="dram", bufs=1, space="DRAM") as dpool,
            tc.tile_pool(name="sbig", bufs=1) as spool,
            tc.tile_pool(name="x", bufs=1) as xpool,
        ):
            # ---- constants (tiny DMAs first) ----
            bt = cpool.tile([P, 6], F32)
            nc.sync.dma_start(bt[:], bias[:])
            mt_ = cpool.tile([P, 3], F32)
            nc.sync.dma_start(mt_[:], msk[:])
            m_top = mt_[:, 0:1]
            m_bot = mt_[:, 1:2]
            coloff = mt_[:, 2:3]
            wc2t = cpool.tile([P, 2, 2, P], F32R)
            nc.sync.dma_start(wc2t[:], wc2[:])

            # ---- PE pre-warm: dummy matmuls on wc2 release the HAM
            # clock gate (~3.4us of activity) while x streams in ----
            wc2f = wc2t[:].rearrange("p a b c -> p (a b c)")
            ptw = psum.tile([P, 512], F32, tag="ps")
            for _ in range(4):
                nc.tensor.matmul(
                    ptw[:, 0:512], wc2f[:, 0:P], wc2f[:, 0:512],
                    start=True, stop=True,
                )

            # ---- weights (phased through 2 slots; p1 first) ----
            def wtile(src):
                t = wpool.tile([P, 2, 9, P], F32R, tag="w")
                nc.sync.dma_start(t[:], src[:])
                return t

            wp1t = wtile(wp1)
            wp2t = wtile(wp2)

            # ---- x slabs, highest rows first (conv p1 runs reverse) ----
            xt = [xpool.tile([P, HS, WP], F32R, name=f"x{s}", tag=f"x{s}") for s in range(2)]
            for r0 in range(HS - 6, -1, -6):
                for s in range(2):
                    nc.sync.dma_start(xt[s][:, r0:r0 + 6, :], xs[s, :, r0:r0 + 6, :])
            xf = [t[:].rearrange("p h w -> p (h w)") for t in xt]
            xsz = HS * WP

            # ---- s slab (conv p1 writes rows 2..65; becomes cp1, then s) ----
            st = spool.tile([P, SS, WP], F32R)
            sf = st[:].rearrange("p h w -> p (h w)")

            NT_A, NCH_A = 416, (OH * WP) // 416

            with tc.tile_pool(name="p2", bufs=1) as p2pool:
                p2t = p2pool.tile([P, OH, WP], F32R)
                p2f = p2t[:].rearrange("p h w -> p (h w)")
                # scan reset mask: 1 everywhere, 0 at the two pad cols of
                # each row (values are post-relu >= 0 so a 0-reset == -inf)
                mk = p2pool.tile([P, PCR, WP], F32)
                nc.vector.memset(mk[:], 1.0)
                nc.vector.tensor_scalar_mul(mk[:, :, 0:1], mk[:, :, 0:1], 0.0)
                nc.vector.tensor_scalar_mul(
                    mk[:, :, WP - 1 : WP], mk[:, :, WP - 1 : WP], 0.0
                )
                mkf = mk[:].rearrange("p h w -> p (h w)")

                def conv_branch(wt_tile, bias_col, out_flat, out_base, order=None):
                    terms = []
                    for s in range(2):
                        for t_i, (ky, kx) in enumerate(TAPS):
                            terms.append(
                                (s, wt_tile[:, s, t_i, :], (1 + ky) * WP + kx)
                            )

                    def epi(ci, pap):
                        ncols = pap.shape[-1]
                        nc.scalar.activation(
                            out_flat[:, out_base + ci * NT_A : out_base + ci * NT_A + ncols],
                            pap,
                            mybir.ActivationFunctionType.Relu,
                            bias=bias_col,
                        )

                    _emit_conv(nc, psum, xf, [xsz, xsz], terms, NT_A, NCH_A, epi,
                               order=order)

                # conv p1 -> s rows 2..65, reverse chunk order
                conv_branch(wp1t, bt[:, 0:1], sf, 2 * WP,
                            order=range(NCH_A - 1, -1, -1))
                # reverse cummax over H (rows 64 -> 2), in place; chain op h
                # only needs the conv chunks covering rows h,h+1, so under
                # reverse chunk order this interleaves with conv p1 on DVE
                for h in range(OH, 1, -1):
                    nc.vector.tensor_tensor(
                        st[:, h, :],
                        st[:, h, :],
                        st[:, h + 1, :],
                        mybir.AluOpType.max,
                    )

                # conv p2: first+last chunks first (they hold the 4 exchange
                # rows), so the collective overlaps the remaining chunks
                conv_branch(wp2t, bt[:, 1:2], p2f, 0,
                            order=[0, NCH_A - 1] + list(range(1, NCH_A - 1)))
                # W-cummax of the 4 exchange rows only (in place, reversed)
                for h in (0, 1, OH - 2, OH - 1):
                    row = p2t[:, h, 1 : WP - 1]
                    nc.vector.tensor_tensor_scan(
                        row[:, ::-1],
                        row[:, ::-1],
                        row[:, ::-1],
                        -1e30,
                        mybir.AluOpType.max,
                        mybir.AluOpType.bypass,
                    )

                # ---- pairwise exchange ----
                # C slots: 0,1 cp1local first2 rows (*mb); 2,3 cp1local last2
                # (*mt); 4,5 cp2 first2 (*mb); 6,7 cp2 last2 (*mt)
                ct = spool_s.tile([P, 8, WP], F32, tag="exch")
                for k, (src, m) in enumerate(
                    [
                        (st[:, 2, :], m_bot),
                        (st[:, 3, :], m_bot),
                        (st[:, 2 + OH - 2, :], m_top),
                        (st[:, 2 + OH - 1, :], m_top),
                        (p2t[:, 0, :], m_bot),
                        (p2t[:, 1, :], m_bot),
                        (p2t[:, OH - 2, :], m_top),
                        (p2t[:, OH - 1, :], m_top),
                    ]
                ):
                    nc.vector.tensor_scalar_mul(ct[:, k, :], src, m)
                cc_in = dpool.tile([P, 8, WP], F32)
                cc_out = dpool.tile([P, 8, WP], F32)
                nc.sync.dma_start(cc_in[:], ct[:])
                nc.gpsimd.collective_compute(
                    "AllReduce",
                    mybir.AluOpType.add,
                    replica_groups=groups,
                    ins=[cc_in[:]],
                    outs=[cc_out[:]],
                )
                rt = spool_s.tile([P, 8, WP], F32R, tag="exch")
                nc.sync.dma_start(rt[:], cc_out[:].bitcast(F32R))

                # u = R[0] + coloff (top cores: partner colmax; bottom: -inf)
                u = spool_s.tile([P, WP], F32R, tag="u")
                nc.vector.tensor_scalar_add(u[:], rt[:, 0, :], coloff)

                # bulk W-cummax: 4 flat reversed scans of 16 rows each with
                # mask-mult state reset at the pad cols (re-scanning the 4
                # exchange rows is idempotent)
                for pc in range(NPC):
                    seg = p2f[:, pc * PCR * WP : (pc + 1) * PCR * WP]
                    nc.vector.tensor_tensor_scan(
                        seg[:, ::-1],
                        seg[:, ::-1],
                        mkf[:, ::-1],
                        0.0,
                        mybir.AluOpType.max,
                        mybir.AluOpType.mult,
                    )

                # per piece: cp1 colmax fixup (DVE), s = cp1+cp2 (gpsimd),
                # zero the pad cols (DVE).  Own colmax (= fixed cp1 row 0)
                # is snapshotted to cm before the add overwrites st row 2.
                cm = spool_s.tile([P, WP], F32R, tag="cm")
                for pc in range(NPC):
                    r0 = 2 + pc * PCR
                    sp = st[:, r0 : r0 + PCR, :]
                    nc.vector.tensor_tensor(
                        sp,
                        sp,
                        u[:, None, :].to_broadcast((P, PCR, WP)),
                        mybir.AluOpType.max,
                    )
                    if pc == 0:
                        nc.vector.tensor_copy(cm[:], st[:, 2, :])
                    nc.gpsimd.tensor_tensor(
                        sp,
                        sp,
                        p2t[:, pc * PCR : pc * PCR + PCR, :],
                        mybir.AluOpType.add,
                    )
                    nc.vector.tensor_scalar_mul(
                        st[:, r0 : r0 + PCR, 0:1], st[:, r0 : r0 + PCR, 0:1], 0.0
                    )
                    nc.vector.tensor_scalar_mul(
                        st[:, r0 : r0 + PCR, WP - 1 : WP],
                        st[:, r0 : r0 + PCR, WP - 1 : WP],
                        0.0,
                    )

                # halo rows of s.  above halo (bottom cores): max(partner
                # cp1local last2, own colmax=st[2]) + partner cp2 last2, *mb
                h0 = spool_s.tile([P, WP], F32R, tag="h0")
                h1 = spool_s.tile([P, WP], F32R, tag="h1")
                for j, dst_row in ((0, 0), (1, 1)):
                    nc.vector.tensor_tensor(
                        h0[:], rt[:, 2 + j, :], cm[:], mybir.AluOpType.max
                    )
                    nc.vector.tensor_tensor(
                        h0[:], h0[:], rt[:, 6 + j, :], mybir.AluOpType.add
                    )
                    nc.vector.tensor_scalar_mul(st[:, dst_row, :], h0[:], m_bot)
                # below halo (top cores): partner cp1local first2 + cp2 first2
                for j, dst_row in ((0, SS - 2), (1, SS - 1)):
                    nc.vector.tensor_tensor(
                        h1[:], rt[:, 0 + j, :], rt[:, 4 + j, :], mybir.AluOpType.add
                    )
                    nc.vector.tensor_scalar_mul(st[:, dst_row, :], h1[:], m_top)
                # zero pad cols of the 4 halo rows (in-place *0: memset may
                # not produce fp32r-matmul operands under this walrus)
                for rr in (0, SS - 2):
                    nc.vector.tensor_scalar_mul(
                        st[:, rr : rr + 2, 0:1], st[:, rr : rr + 2, 0:1], 0.0
                    )
                    nc.vector.tensor_scalar_mul(
                        st[:, rr : rr + 2, WP - 1 : WP],
                        st[:, rr : rr + 2, WP - 1 : WP],
                        0.0,
                    )

            # ---- conv c1 + c2 -> r_ext ----
            wc1t = wtile(wc1)
            NT_C, NCH_C = 390, ((OH + 2) * WP) // 390
            ssz = SS * WP
            with tc.tile_pool(name="r", bufs=1) as rpool:
                rt2 = [rpool.tile([P, HS, WP], F32R, name=f"r{i}", tag=f"r{i}") for i in range(2)]
                rf = [t[:].rearrange("p h w -> p (h w)") for t in rt2]
                for oh_half in range(2):
                    terms = []
                    for t_i, (ky, kx) in enumerate(TAPS):
                        terms.append((0, wc1t[:, oh_half, t_i, :], (1 + ky) * WP + kx))
                    for s in range(2):
                        terms.append((1 + s, wc2t[:, s, oh_half, :], 0))

                    def epi(ci, pap, oh_half=oh_half):
                        ncols = pap.shape[-1]
                        nc.scalar.activation(
                            rf[oh_half][:, ci * NT_C : ci * NT_C + ncols],
                            pap,
                            mybir.ActivationFunctionType.Relu,
                            bias=bt[:, 2 + oh_half : 3 + oh_half],
                        )

                    _emit_conv(
                        nc, psum, [sf, xf[0], xf[1]], [ssz, xsz, xsz],
                        terms, NT_C, NCH_C, epi,
                    )
                    # mask invalid halo rows, zero pad cols (per half so the
                    # ops overlap the other half's matmuls)
                    t = rt2[oh_half]
                    nc.vector.tensor_scalar_mul(t[:, 0, :], t[:, 0, :], m_bot)
                    nc.vector.tensor_scalar_mul(
                        t[:, HS - 1, :], t[:, HS - 1, :], m_top
                    )
                    nc.vector.tensor_scalar_mul(t[:, :, 0:1], t[:, :, 0:1], 0.0)
                    nc.vector.tensor_scalar_mul(
                        t[:, :, WP - 1 : WP], t[:, :, WP - 1 : WP], 0.0
                    )

                # ---- conv p3 -> out ----
                wp3a = wpool.tile([P, 2, 9, P], F32R, tag="w")
                nc.sync.dma_start(wp3a[:], wp3[:, 0])
                wp3b = wpool.tile([P, 2, 9, P], F32R, tag="w")
                nc.sync.dma_start(wp3b[:], wp3[:, 1])
                NT_O, NCH_O = 260, (OH * WP) // 260
                rsz = HS * WP
                for oh_half, wtile_ in ((0, wp3a), (1, wp3b)):
                    terms = []
                    for s in range(2):
                        for t_i, (ky, kx) in enumerate(TAPS):
                            terms.append(
                                (s, wtile_[:, s, t_i, :], (1 + ky) * WP + kx)
                            )

                    def epi(ci, pap, oh_half=oh_half):
                        ncols = pap.shape[-1]
                        ot = opool.tile([P, 2, WP], F32, tag="ost")
                        nc.scalar.activation(
                            ot[:].rearrange("p r w -> p (r w)")[:, :ncols],
                            pap,
                            mybir.ActivationFunctionType.Relu,
                            bias=bt[:, 4 + oh_half : 5 + oh_half],
                        )
                        nc.sync.dma_start(
                            o[oh_half, :, ci * 2 : ci * 2 + 2, :],
                            ot[:, :, 1 : WP - 1],
                        )

                    _emit_conv(nc, psum, rf, [rsz, rsz], terms, NT_O, NCH_O, epi)

    _fix_multiwaits(nc)
    return nc


_NC = None


def _get_nc():
    global _NC
    if _NC is None:
        _NC = build_nc()
    return _NC


def _fold_bn(w, g, b, m, v):
    s = (g / np.sqrt(v + EPS)).astype(np.float32)
    t = (b - m * s).astype(np.float32)
    return w * s[:, None, None, None], t


def kernel(**inputs):
    x = np.asarray(inputs["x"], np.float32)

    w_p1, t_p1 = _fold_bn(
        np.asarray(inputs["w_p1"], np.float32),
        inputs["g_p1"], inputs["b_p1"], inputs["m_p1"], inputs["v_p1"],
    )
    w_p2, t_p2 = _fold_bn(
        np.asarray(inputs["w_p2"], np.float32),
        inputs["g_p2"], inputs["b_p2"], inputs["m_p2"], inputs["v_p2"],
    )
    w_c1, t_c1 = _fold_bn(
        np.asarray(inputs["w_c1"], np.float32),
        inputs["g_c1"], inputs["b_c1"], inputs["m_c1"], inputs["v_c1"],
    )
    w_c2, t_c2 = _fold_bn(
        np.asarray(inputs["w_c2"], np.float32),
        inputs["g_c2"], inputs["b_c2"], inputs["m_c2"], inputs["v_c2"],
    )
    w_p3, t_p3 = _fold_bn(
        np.asarray(inputs["w_p3"], np.float32),
        inputs["g_p3"], inputs["b_p3"], inputs["m_p3"], inputs["v_p3"],
    )

    # weight layouts (see build_nc): contraction channel on partitions
    def lay3x3(wf, cout_half):
        # wf [O, I, 3, 3] -> [128ci, n_i_sub, (oh?), 9, 128co]
        O, I = wf.shape[0], wf.shape[1]
        a = wf.reshape(O // P, P, I // P, P, 3, 3)  # [ohs, co, s, ci, ky, kx]
        a = a.transpose(3, 0, 2, 4, 5, 1)  # [ci, ohs, s, ky, kx, co]
        return np.ascontiguousarray(a)

    wp1a = lay3x3(w_p1, 1).reshape(P, 1, 2, 9, P)[:, 0]          # [128,2,9,128]
    wp2a = lay3x3(w_p2, 1).reshape(P, 1, 2, 9, P)[:, 0]
    wc1a = lay3x3(w_c1, 2).reshape(P, 2, 1, 9, P)[:, :, 0]       # [128,2oh,9,128]
    wp3a = lay3x3(w_p3, 2)                                        # [128,2oh,2s,9,128]
    wc2a = np.ascontiguousarray(
        w_c2[:, :, 0, 0].reshape(2, P, 2, P).transpose(3, 2, 0, 1)
    )  # [128ci, 2s, 2oh, 128co]

    bias = np.zeros((P, 6), np.float32)
    bias[:, 0] = t_p1
    bias[:, 1] = t_p2
    bc = t_c1 + t_c2
    bias[:, 2] = bc[:P]
    bias[:, 3] = bc[P:]
    bias[:, 4] = t_p3[:P]
    bias[:, 5] = t_p3[P:]

    # x slabs per core, W-padded + H halo, rounded to f32r
    xr = x.reshape(B, 2, P, H, W)
    slabs = np.zeros((B, 2, 2, P, HS, WP), np.float32)  # [b, half, s, p, h, w]
    for b in range(B):
        slabs[b, 0, :, :, 1:HS, 1 : WP - 1] = xr[b, :, :, 0:65, :]
        slabs[b, 1, :, :, 0 : HS - 1, 1 : WP - 1] = xr[b, :, :, 63:128, :]
    slabs = _r12(slabs)

    wmaps = {
        "wp1": _r12(wp1a),
        "wp2": _r12(wp2a),
        "wc1": _r12(wc1a),
        "wc2": _r12(wc2a),
        "wp3": _r12(wp3a),
        "bias": bias,
    }
    in_maps = []
    for b in range(B):
        for half in range(2):
            msk = np.zeros((P, 3), np.float32)
            if half == 0:  # top
                msk[:, 0] = 1.0  # m_top
                msk[:, 2] = 0.0
            else:  # bottom
                msk[:, 1] = 1.0  # m_bot
                msk[:, 2] = -1e30
            in_maps.append({"xs": slabs[b, half], "msk": msk, **wmaps})

    global _last_in_maps
    _last_in_maps = in_maps

    nc = _get_nc()
    res = run_bass_kernel_spmd(nc, in_maps, list(range(8)))

    out = np.empty((B, CIN, H, W), np.float32)
    for b in range(B):
        out[b, :, 0:OH] = res.results[2 * b]["o"].reshape(CIN, OH, W)
        out[b, :, OH:H] = res.results[2 * b + 1]["o"].reshape(CIN, OH, W)
    return out


if __name__ == "__main__":
    import reference

    inp = {k: np.asarray(v) for k, v in reference.setup_inputs().items()}
    exp = np.asarray(reference.reference(**inp))
    got = kernel(**inp)
    err = np.abs(got - exp)
    rel = err.max() / max(np.abs(exp).max(), 1e-6)
    print("abs err max:", err.max(), "rel (vs absmax):", rel)


# revision 8
# speedup vs baseline: 1.2702x; 1.0684x over previous
"""Corner-pooling module kernel for 8 Trainium2 NeuronCores.

Reference computation (NCHW, fp32):
    p1 = relu(bn(conv3x3(x, w_p1)))          # 256 -> 128 ch
    p2 = relu(bn(conv3x3(x, w_p2)))          # 256 -> 128 ch
    cp1 = cummax(p1, axis=H, reverse=True)
    cp2 = cummax(p2, axis=W, reverse=True)
    r  = relu(bn(conv3x3(cp1+cp2, w_c1)) + bn(conv1x1(x, w_c2)))
    out = relu(bn(conv3x3(r, w_p3)))

Sharding: 8 cores = 4 samples x 2 H-halves (core 2b: rows 0..63 of
sample b, core 2b+1: rows 64..127).  All BN scales are folded into the
conv weights host-side (per-out-channel bias remains), so every conv is
matmul-accumulate + one ACT relu-with-bias epilogue.  Convs run as
flat-chunked matmuls over a W-padded (130-wide) layout with channels on
partitions, in float32r (full-rate PE, ~2.4e-4 eps).

Schedule (the point of this version): conv p1 runs FIRST in reverse
chunk order so the 63-step reverse H-cummax chain interleaves under it
on DVE; conv p2 runs second with its first+last chunks prioritized so
the 8-row pairwise AllReduce (the only cross-core dependency) plus all
cummax fixups overlap p2's remaining matmuls.  The W-cummax runs as 4
flat multi-row reversed scans using a (max,mult) mask-reset trick, and
the colmax-broadcast fixup / s=cp1+cp2 add / pad zeroing are split into
4 row pieces (fixups on DVE, adds on gpsimd) so conv c1 starts the
moment p2's matmuls end.  x streams highest-rows-first so conv p1's
first (reverse) chunk starts ~4us in, and a few dummy matmuls on the
wc2 tile pre-warm the PE clock gate during the initial DMA.
"""

import sys

sys.path.insert(0, "/opt/trn_rl_repo")

import numpy as np

import bass_rust
import concourse.bass as bass
import concourse.mybir as mybir
from concourse.bass_utils import run_bass_kernel_spmd
from concourse.tile import TileContext

F32 = mybir.dt.float32
F32R = mybir.dt.float32r
EPS = 1e-5

B, CIN, H, W = 4, 256, 128, 128
MID = 128
P = 128
WP = W + 2          # padded width
HS = 66             # x-slab rows: own 64 + 1 halo above + 1 below
SS = 68             # s-slab rows: own 64 + 2 halo above + 2 below
OH = 64             # own rows per core
TAPS = [(ky, kx) for ky in (-1, 0, 1) for kx in (-1, 0, 1)]
NPC = 4             # fixup/add/scan pieces
PCR = OH // NPC     # rows per piece


def _r12(x):
    """Round fp32 to float32r precision (11 explicit mantissa bits, RNE)."""
    u = np.ascontiguousarray(x, dtype=np.float32).view(np.uint32)
    r = (u + 0x7FF + ((u >> 12) & 1)) & 0xFFFFF000
    return r.view(np.float32)


def _fix_multiwaits(nc):
    """walrus in this container accepts at most ONE sem wait per
    instruction; split extras onto same-engine nops placed just before."""

    def steal_nop(eng):
        bi = nc.engines[eng].nop()
        ins = bi.ins
        cur = nc.cur_bb.bb
        lst = cur.instructions
        assert lst[-1] is ins or lst[-1].name == ins.name
        cur.instructions = lst[:-1]
        return ins

    for fn in nc.m.functions:
        for bb in fn.blocks:
            out = []
            changed = False
            for inst in bb.instructions:
                si = inst.sync_info
                waits = list(si.on_wait) if si is not None and si.on_wait else []
                if len(waits) > 1:
                    for wv in waits[:-1]:
                        nop = steal_nop(inst.engine)
                        nop.sync_info = bass_rust.SyncInfo(on_wait=[wv], on_update=[])
                        out.append(nop)
                    inst.sync_info = bass_rust.SyncInfo(
                        on_wait=[waits[-1]], on_update=list(si.on_update or [])
                    )
                    changed = True
                out.append(inst)
            if changed:
                bb.instructions = out
    return nc


def _emit_conv(nc, psum, rhs_flats, rhs_sizes, terms, nt, nchunks, epilogue,
               order=None):
    """Flat-chunked conv: for each output chunk accumulate all terms'
    matmuls in one PSUM tile, then run epilogue(chunk_idx, psum_ap).

    terms: list of (rhs_idx, lhsT_ap, off) where the term reads
    rhs_flats[rhs_idx][:, q+off : q+off+nt] for out positions [q, q+nt).
    Edge terms are trimmed by one element at the slab ends.
    order: chunk iteration order (default ascending).
    """
    for ci in (range(nchunks) if order is None else order):
        q = ci * nt
        full, part = [], []
        for rhs_idx, lhsT, off in terms:
            o = q + off
            lo = max(0, -o)
            hi = min(nt, rhs_sizes[rhs_idx] - o)
            # fp32r matmul dst needs even offset+length; the extra trimmed
            # elements only ever read zero pad positions
            lo += lo & 1
            hi -= (hi - lo) & 1
            (full if (lo == 0 and hi == nt) else part).append(
                (rhs_idx, lhsT, o, lo, hi)
            )
        seq = [full[0]] + part + full[1:]
        pt = psum.tile([P, 512], F32, tag="ps")
        n = len(seq)
        for i, (rhs_idx, lhsT, o, lo, hi) in enumerate(seq):
            nc.tensor.matmul(
                pt[:, lo:hi],
                lhsT,
                rhs_flats[rhs_idx][:, o + lo : o + hi],
                start=(i == 0),
                stop=(i == n - 1),
            )
        epilogue(ci, pt[:, :nt])


def build_nc():
    nc = bass.Bass()

    xs = nc.dram_tensor("xs", [2, P, HS, WP], F32R, kind="ExternalInput")
    wp1 = nc.dram_tensor("wp1", [P, 2, 9, P], F32R, kind="ExternalInput")
    wp2 = nc.dram_tensor("wp2", [P, 2, 9, P], F32R, kind="ExternalInput")
    wc1 = nc.dram_tensor("wc1", [P, 2, 9, P], F32R, kind="ExternalInput")
    wc2 = nc.dram_tensor("wc2", [P, 2, 2, P], F32R, kind="ExternalInput")
    wp3 = nc.dram_tensor("wp3", [P, 2, 2, 9, P], F32R, kind="ExternalInput")
    bias = nc.dram_tensor("bias", [P, 6], F32, kind="ExternalInput")
    msk = nc.dram_tensor("msk", [P, 3], F32, kind="ExternalInput")
    o = nc.dram_tensor("o", [2, P, OH, W], F32, kind="ExternalOutput")

    groups = [[0, 1], [2, 3], [4, 5], [6, 7]]

    with TileContext(nc) as tc:
        with (
            tc.tile_pool(name="const", bufs=1) as cpool,
            tc.tile_pool(name="wt", bufs=2) as wpool,
            tc.tile_pool(name="psum", bufs=6, space="PSUM") as psum,
            tc.tile_pool(name="scratch", bufs=1) as spool_s,
            tc.tile_pool(name="ost", bufs=3) as opool,
            tc.tile_pool(name="dram", bufs=1, space="DRAM") as dpool,
            tc.tile_pool(name="sbig", bufs=1) as spool,
            tc.tile_pool(name="x", bufs=1) as xpool,
        ):
            # ---- constants (tiny DMAs first) ----
            bt = cpool.tile([P, 6], F32)
            nc.sync.dma_start(bt[:], bias[:])
            mt_ = cpool.tile([P, 3], F32)
            nc.sync.dma_start(mt_[:], msk[:])
            m_top = mt_[:, 0:1]
            m_bot = mt_[:, 1:2]
            coloff = mt_[:, 2:3]
            wc2t = cpool.tile([P, 2, 2, P], F32R)
            nc.sync.dma_start(wc2t[:], wc2[:])

            # ---- PE pre-warm: dummy matmuls on wc2 release the HAM
            # clock gate (~3.4us of activity) while x streams in ----
            wc2f = wc2t[:].rearrange("p a b c -> p (a b c)")
            ptw = psum.tile([P, 512], F32, tag="ps")
            for _ in range(3):
                nc.tensor.matmul(
                    ptw[:, 0:512], wc2f[:, 0:P], wc2f[:, 0:512],
                    start=True, stop=True,
                )

            # ---- weights: slot order wp1,wp2 (so wc1/wp3a later reuse the
            # slot freed at p1/p2 end) but wp2's DMA issues first — the two
            # early p2 chunks need it before conv p1 starts ----
            wp1t = wpool.tile([P, 2, 9, P], F32R, tag="w")
            wp2t = wpool.tile([P, 2, 9, P], F32R, tag="w")
            nc.sync.dma_start(wp2t[:], wp2[:])

            def wtile(src):
                t = wpool.tile([P, 2, 9, P], F32R, tag="w")
                nc.sync.dma_start(t[:], src[:])
                return t

            # ---- x slabs, highest rows first (conv p1 runs reverse); the
            # rows-0 piece comes third so p2's chunk-0 (exchange rows 0,1)
            # can run early too ----
            xt = [xpool.tile([P, HS, WP], F32R, name=f"x{s}", tag=f"x{s}") for s in range(2)]
            piece_order = [HS - 6, HS - 12, 0] + list(range(HS - 18, 0, -6))
            for k, r0 in enumerate(piece_order):
                for s in range(2):
                    nc.sync.dma_start(xt[s][:, r0:r0 + 6, :], xs[s, :, r0:r0 + 6, :])
                if k == 0:
                    nc.sync.dma_start(wp1t[:], wp1[:])
            xf = [t[:].rearrange("p h w -> p (h w)") for t in xt]
            xsz = HS * WP

            # ---- s slab (conv p1 writes rows 2..65; becomes cp1, then s) ----
            st = spool.tile([P, SS, WP], F32R)
            sf = st[:].rearrange("p h w -> p (h w)")

            NT_A, NCH_A = 416, (OH * WP) // 416

            with tc.tile_pool(name="p2", bufs=1) as p2pool:
                p2t = p2pool.tile([P, OH, WP], F32R)
                p2f = p2t[:].rearrange("p h w -> p (h w)")
                # scan reset mask: 1 everywhere, 0 at the two pad cols of
                # each row (values are post-relu >= 0 so a 0-reset == -inf)
                mk = p2pool.tile([P, PCR, WP], F32)
                nc.vector.memset(mk[:], 1.0)
                nc.vector.tensor_scalar_mul(mk[:, :, 0:1], mk[:, :, 0:1], 0.0)
                nc.vector.tensor_scalar_mul(
                    mk[:, :, WP - 1 : WP], mk[:, :, WP - 1 : WP], 0.0
                )
                mkf = mk[:].rearrange("p h w -> p (h w)")

                def conv_branch(wt_tile, bias_col, out_flat, out_base, order=None):
                    terms = []
                    for s in range(2):
                        for t_i, (ky, kx) in enumerate(TAPS):
                            terms.append(
                                (s, wt_tile[:, s, t_i, :], (1 + ky) * WP + kx)
                            )

                    def epi(ci, pap):
                        ncols = pap.shape[-1]
                        nc.scalar.activation(
                            out_flat[:, out_base + ci * NT_A : out_base + ci * NT_A + ncols],
                            pap,
                            mybir.ActivationFunctionType.Relu,
                            bias=bias_col,
                        )

                    _emit_conv(nc, psum, xf, [xsz, xsz], terms, NT_A, NCH_A, epi,
                               order=order)

                # p2's exchange chunks (rows 62,63 and 0,1) run before conv
                # p1 so the collective can fire the moment the chain ends
                conv_branch(wp2t, bt[:, 1:2], p2f, 0, order=[NCH_A - 1, 0])
                # W-cummax of the 4 exchange rows (in place, reversed)
                for h in (OH - 2, OH - 1, 0, 1):
                    row = p2t[:, h, 1 : WP - 1]
                    nc.vector.tensor_tensor_scan(
                        row[:, ::-1],
                        row[:, ::-1],
                        row[:, ::-1],
                        -1e30,
                        mybir.AluOpType.max,
                        mybir.AluOpType.bypass,
                    )

                # conv p1 -> s rows 2..65, reverse chunk order
                conv_branch(wp1t, bt[:, 0:1], sf, 2 * WP,
                            order=range(NCH_A - 1, -1, -1))
                # reverse cummax over H (rows 64 -> 2), in place; chain op h
                # only needs the conv chunks covering rows h,h+1, so under
                # reverse chunk order this interleaves with conv p1 on DVE
                for h in range(OH, 1, -1):
                    nc.vector.tensor_tensor(
                        st[:, h, :],
                        st[:, h, :],
                        st[:, h + 1, :],
                        mybir.AluOpType.max,
                    )

                # ---- pairwise exchange ----
                # C slots: 0,1 cp1local first2 rows (*mb); 2,3 cp1local last2
                # (*mt); 4,5 cp2 first2 (*mb); 6,7 cp2 last2 (*mt).
                # bf16 payload: halves the collective time; only halo rows
                # and the colmax are affected (~0.4% rounding, tol is 2e-2).
                BF16 = mybir.dt.bfloat16
                ct = spool_s.tile([P, 8, WP], BF16, tag="exch")
                for k, (src, m) in enumerate(
                    [
                        (st[:, 2, :], m_bot),
                        (st[:, 3, :], m_bot),
                        (st[:, 2 + OH - 2, :], m_top),
                        (st[:, 2 + OH - 1, :], m_top),
                        (p2t[:, 0, :], m_bot),
                        (p2t[:, 1, :], m_bot),
                        (p2t[:, OH - 2, :], m_top),
                        (p2t[:, OH - 1, :], m_top),
                    ]
                ):
                    nc.vector.tensor_scalar_mul(ct[:, k, :], src, m)
                cc_in = dpool.tile([P, 8, WP], BF16)
                cc_out = dpool.tile([P, 8, WP], BF16)
                nc.sync.dma_start(cc_in[:], ct[:])
                nc.gpsimd.collective_compute(
                    "AllReduce",
                    mybir.AluOpType.add,
                    replica_groups=groups,
                    ins=[cc_in[:]],
                    outs=[cc_out[:]],
                )
                rt = spool_s.tile([P, 8, WP], BF16, tag="exch")
                nc.sync.dma_start(rt[:], cc_out[:])

                # u = R[0] + coloff (top cores: partner colmax; bottom: -inf)
                u = spool_s.tile([P, WP], F32R, tag="u")
                nc.vector.tensor_scalar_add(u[:], rt[:, 0, :], coloff)

                # conv p2's remaining chunks run while the collective is in
                # flight and the pieces below drain
                conv_branch(wp2t, bt[:, 1:2], p2f, 0,
                            order=range(1, NCH_A - 1))

                # per 16-row piece: bulk W-cummax (flat reversed scan with
                # mask-mult state reset at the pad cols; re-scanning the
                # exchange rows is idempotent), cp1 colmax fixup, s=cp1+cp2,
                # pad-col zeroing — piece 0 first so conv c1's first chunks
                # are unblocked before conv p2 even finishes.
                cm = spool_s.tile([P, WP], F32R, tag="cm")
                h0 = spool_s.tile([P, WP], F32R, tag="h0")
                h1 = spool_s.tile([P, WP], F32R, tag="h1")
                for pc in range(NPC):
                    seg = p2f[:, pc * PCR * WP : (pc + 1) * PCR * WP]
                    nc.vector.tensor_tensor_scan(
                        seg[:, ::-1],
                        seg[:, ::-1],
                        mkf[:, ::-1],
                        0.0,
                        mybir.AluOpType.max,
                        mybir.AluOpType.mult,
                    )
                    r0 = 2 + pc * PCR
                    sp = st[:, r0 : r0 + PCR, :]
                    nc.vector.tensor_tensor(
                        sp,
                        sp,
                        u[:, None, :].to_broadcast((P, PCR, WP)),
                        mybir.AluOpType.max,
                    )
                    if pc == 0:
                        # own colmax (= fixed cp1 row 0), snapshotted before
                        # the add overwrites st row 2
                        nc.vector.tensor_copy(cm[:], st[:, 2, :])
                    nc.vector.tensor_tensor(
                        sp,
                        sp,
                        p2t[:, pc * PCR : pc * PCR + PCR, :],
                        mybir.AluOpType.add,
                    )
                    nc.vector.tensor_scalar_mul(
                        st[:, r0 : r0 + PCR, 0:1], st[:, r0 : r0 + PCR, 0:1], 0.0
                    )
                    nc.vector.tensor_scalar_mul(
                        st[:, r0 : r0 + PCR, WP - 1 : WP],
                        st[:, r0 : r0 + PCR, WP - 1 : WP],
                        0.0,
                    )
                    if pc == 0:
                        # halo rows right after piece 0 (conv c1 chunk 0
                        # needs them).  above halo (bottom cores):
                        # max(partner cp1local last2, own colmax) + partner
                        # cp2 last2, *mb
                        for j, dst_row in ((0, 0), (1, 1)):
                            nc.vector.tensor_tensor(
                                h0[:], rt[:, 2 + j, :], cm[:], mybir.AluOpType.max
                            )
                            nc.vector.tensor_tensor(
                                h0[:], h0[:], rt[:, 6 + j, :], mybir.AluOpType.add
                            )
                            nc.vector.tensor_scalar_mul(
                                st[:, dst_row, :], h0[:], m_bot
                            )
                        # below halo (top cores): partner cp1local first2 +
                        # cp2 first2
                        for j, dst_row in ((0, SS - 2), (1, SS - 1)):
                            nc.vector.tensor_tensor(
                                h1[:], rt[:, 0 + j, :], rt[:, 4 + j, :],
                                mybir.AluOpType.add,
                            )
                            nc.vector.tensor_scalar_mul(
                                st[:, dst_row, :], h1[:], m_top
                            )
                        # zero pad cols of the 4 halo rows (in-place *0:
                        # memset may not produce fp32r-matmul operands under
                        # this walrus)
                        for rr in (0, SS - 2):
                            nc.vector.tensor_scalar_mul(
                                st[:, rr : rr + 2, 0:1],
                                st[:, rr : rr + 2, 0:1],
                                0.0,
                            )
                            nc.vector.tensor_scalar_mul(
                                st[:, rr : rr + 2, WP - 1 : WP],
                                st[:, rr : rr + 2, WP - 1 : WP],
                                0.0,
                            )

            # ---- conv c1 + c2 -> r_ext ----
            wc1t = wtile(wc1)
            NT_C, NCH_C = 390, ((OH + 2) * WP) // 390
            ssz = SS * WP
            with tc.tile_pool(name="r", bufs=1) as rpool:
                rt2 = [rpool.tile([P, HS, WP], F32R, name=f"r{i}", tag=f"r{i}") for i in range(2)]
                rf = [t[:].rearrange("p h w -> p (h w)") for t in rt2]
                for oh_half in range(2):
                    terms = []
                    for t_i, (ky, kx) in enumerate(TAPS):
                        terms.append((0, wc1t[:, oh_half, t_i, :], (1 + ky) * WP + kx))
                    for s in range(2):
                        terms.append((1 + s, wc2t[:, s, oh_half, :], 0))

                    def epi(ci, pap, oh_half=oh_half):
                        ncols = pap.shape[-1]
                        nc.scalar.activation(
                            rf[oh_half][:, ci * NT_C : ci * NT_C + ncols],
                            pap,
                            mybir.ActivationFunctionType.Relu,
                            bias=bt[:, 2 + oh_half : 3 + oh_half],
                        )

                    _emit_conv(
                        nc, psum, [sf, xf[0], xf[1]], [ssz, xsz, xsz],
                        terms, NT_C, NCH_C, epi,
                    )
                    # mask invalid halo rows, zero pad cols (per half so the
                    # ops overlap the other half's matmuls)
                    t = rt2[oh_half]
                    nc.vector.tensor_scalar_mul(t[:, 0, :], t[:, 0, :], m_bot)
                    nc.vector.tensor_scalar_mul(
                        t[:, HS - 1, :], t[:, HS - 1, :], m_top
                    )
                    nc.vector.tensor_scalar_mul(t[:, :, 0:1], t[:, :, 0:1], 0.0)
                    nc.vector.tensor_scalar_mul(
                        t[:, :, WP - 1 : WP], t[:, :, WP - 1 : WP], 0.0
                    )

                # ---- conv p3 -> out ----
                wp3a = wpool.tile([P, 2, 9, P], F32R, tag="w")
                nc.sync.dma_start(wp3a[:], wp3[:, 0])
                wp3b = wpool.tile([P, 2, 9, P], F32R, tag="w")
                nc.sync.dma_start(wp3b[:], wp3[:, 1])
                NT_O, NCH_O = 260, (OH * WP) // 260
                rsz = HS * WP
                for oh_half, wtile_ in ((0, wp3a), (1, wp3b)):
                    terms = []
                    for s in range(2):
                        for t_i, (ky, kx) in enumerate(TAPS):
                            terms.append(
                                (s, wtile_[:, s, t_i, :], (1 + ky) * WP + kx)
                            )

                    def epi(ci, pap, oh_half=oh_half):
                        ncols = pap.shape[-1]
                        ot = opool.tile([P, 2, WP], F32, tag="ost")
                        nc.scalar.activation(
                            ot[:].rearrange("p r w -> p (r w)")[:, :ncols],
                            pap,
                            mybir.ActivationFunctionType.Relu,
                            bias=bt[:, 4 + oh_half : 5 + oh_half],
                        )
                        nc.sync.dma_start(
                            o[oh_half, :, ci * 2 : ci * 2 + 2, :],
                            ot[:, :, 1 : WP - 1],
                        )

                    _emit_conv(nc, psum, rf, [rsz, rsz], terms, NT_O, NCH_O, epi)

    _fix_multiwaits(nc)
    return nc


_NC = None


def _get_nc():
    global _NC
    if _NC is None:
        _NC = build_nc()
    return _NC


def _fold_bn(w, g, b, m, v):
    s = (g / np.sqrt(v + EPS)).astype(np.float32)
    t = (b - m * s).astype(np.float32)
    return w * s[:, None, None, None], t


def kernel(**inputs):
    x = np.asarray(inputs["x"], np.float32)

    w_p1, t_p1 = _fold_bn(
        np.asarray(inputs["w_p1"], np.float32),
        inputs["g_p1"], inputs["b_p1"], inputs["m_p1"], inputs["v_p1"],
    )
    w_p2, t_p2 = _fold_bn(
        np.asarray(inputs["w_p2"], np.float32),
        inputs["g_p2"], inputs["b_p2"], inputs["m_p2"], inputs["v_p2"],
    )
    w_c1, t_c1 = _fold_bn(
        np.asarray(inputs["w_c1"], np.float32),
        inputs["g_c1"], inputs["b_c1"], inputs["m_c1"], inputs["v_c1"],
    )
    w_c2, t_c2 = _fold_bn(
        np.asarray(inputs["w_c2"], np.float32),
        inputs["g_c2"], inputs["b_c2"], inputs["m_c2"], inputs["v_c2"],
    )
    w_p3, t_p3 = _fold_bn(
        np.asarray(inputs["w_p3"], np.float32),
        inputs["g_p3"], inputs["b_p3"], inputs["m_p3"], inputs["v_p3"],
    )

    # weight layouts (see build_nc): contraction channel on partitions
    def lay3x3(wf, cout_half):
        # wf [O, I, 3, 3] -> [128ci, n_i_sub, (oh?), 9, 128co]
        O, I = wf.shape[0], wf.shape[1]
        a = wf.reshape(O // P, P, I // P, P, 3, 3)  # [ohs, co, s, ci, ky, kx]
        a = a.transpose(3, 0, 2, 4, 5, 1)  # [ci, ohs, s, ky, kx, co]
        return np.ascontiguousarray(a)

    wp1a = lay3x3(w_p1, 1).reshape(P, 1, 2, 9, P)[:, 0]          # [128,2,9,128]
    wp2a = lay3x3(w_p2, 1).reshape(P, 1, 2, 9, P)[:, 0]
    wc1a = lay3x3(w_c1, 2).reshape(P, 2, 1, 9, P)[:, :, 0]       # [128,2oh,9,128]
    wp3a = lay3x3(w_p3, 2)                                        # [128,2oh,2s,9,128]
    wc2a = np.ascontiguousarray(
        w_c2[:, :, 0, 0].reshape(2, P, 2, P).transpose(3, 2, 0, 1)
    )  # [128ci, 2s, 2oh, 128co]

    bias = np.zeros((P, 6), np.float32)
    bias[:, 0] = t_p1
    bias[:, 1] = t_p2
    bc = t_c1 + t_c2
    bias[:, 2] = bc[:P]
    bias[:, 3] = bc[P:]
    bias[:, 4] = t_p3[:P]
    bias[:, 5] = t_p3[P:]

    # x slabs per core, W-padded + H halo, rounded to f32r
    xr = x.reshape(B, 2, P, H, W)
    slabs = np.zeros((B, 2, 2, P, HS, WP), np.float32)  # [b, half, s, p, h, w]
    for b in range(B):
        slabs[b, 0, :, :, 1:HS, 1 : WP - 1] = xr[b, :, :, 0:65, :]
        slabs[b, 1, :, :, 0 : HS - 1, 1 : WP - 1] = xr[b, :, :, 63:128, :]
    slabs = _r12(slabs)

    wmaps = {
        "wp1": _r12(wp1a),
        "wp2": _r12(wp2a),
        "wc1": _r12(wc1a),
        "wc2": _r12(wc2a),
        "wp3": _r12(wp3a),
        "bias": bias,
    }
    in_maps = []
    for b in range(B):
        for half in range(2):
            msk = np.zeros((P, 3), np.float32)
            if half == 0:  # top
                msk[:, 0] = 1.0  # m_top
                msk[:, 2] = 0.0
            else:  # bottom
                msk[:, 1] = 1.0  # m_bot
                msk[:, 2] = -1e30
            in_maps.append({"xs": slabs[b, half], "msk": msk, **wmaps})

    global _last_in_maps
    _last_in_maps = in_maps

    nc = _get_nc()
    res = run_bass_kernel_spmd(nc, in_maps, list(range(8)))

    out = np.empty((B, CIN, H, W), np.float32)
    for b in range(B):
        out[b, :, 0:OH] = res.results[2 * b]["o"].reshape(CIN, OH, W)
        out[b, :, OH:H] = res.results[2 * b + 1]["o"].reshape(CIN, OH, W)
    return out


if __name__ == "__main__":
    import reference

    inp = {k: np.asarray(v) for k, v in reference.setup_inputs().items()}
    exp = np.asarray(reference.reference(**inp))
    got = kernel(**inp)
    err = np.abs(got - exp)
    rel = err.max() / max(np.abs(exp).max(), 1e-6)
    print("abs err max:", err.max(), "rel (vs absmax):", rel)


# revision 11
# speedup vs baseline: 1.2774x; 1.0056x over previous
"""Corner-pooling module kernel for 8 Trainium2 NeuronCores.

Reference computation (NCHW, fp32):
    p1 = relu(bn(conv3x3(x, w_p1)))          # 256 -> 128 ch
    p2 = relu(bn(conv3x3(x, w_p2)))          # 256 -> 128 ch
    cp1 = cummax(p1, axis=H, reverse=True)
    cp2 = cummax(p2, axis=W, reverse=True)
    r  = relu(bn(conv3x3(cp1+cp2, w_c1)) + bn(conv1x1(x, w_c2)))
    out = relu(bn(conv3x3(r, w_p3)))

Sharding: 8 cores = 4 samples x 2 H-halves (core 2b: rows 0..63 of
sample b, core 2b+1: rows 64..127).  All BN scales are folded into the
conv weights host-side (per-out-channel bias remains), so every conv is
matmul-accumulate + one ACT relu-with-bias epilogue.  Convs run as
flat-chunked matmuls over a W-padded (130-wide) layout with channels on
partitions, in float32r (full-rate PE, ~2.4e-4 eps).

Schedule (the point of this version): conv p1 runs FIRST in reverse
chunk order so the 63-step reverse H-cummax chain interleaves under it
on DVE; conv p2 runs second with its first+last chunks prioritized so
the 8-row pairwise AllReduce (the only cross-core dependency) plus all
cummax fixups overlap p2's remaining matmuls.  The W-cummax runs as 4
flat multi-row reversed scans using a (max,mult) mask-reset trick, and
the colmax-broadcast fixup / s=cp1+cp2 add / pad zeroing are split into
4 row pieces (fixups on DVE, adds on gpsimd) so conv c1 starts the
moment p2's matmuls end.  x streams highest-rows-first so conv p1's
first (reverse) chunk starts ~4us in, and a few dummy matmuls on the
wc2 tile pre-warm the PE clock gate during the initial DMA.
"""

import sys

sys.path.insert(0, "/opt/trn_rl_repo")

import numpy as np

import bass_rust
import concourse.bass as bass
import concourse.mybir as mybir
from concourse.bass_utils import run_bass_kernel_spmd
from concourse.tile import TileContext

F32 = mybir.dt.float32
F32R = mybir.dt.float32r
EPS = 1e-5

B, CIN, H, W = 4, 256, 128, 128
MID = 128
P = 128
WP = W + 2          # padded width
HS = 66             # x-slab rows: own 64 + 1 halo above + 1 below
SS = 68             # s-slab rows: own 64 + 2 halo above + 2 below
OH = 64             # own rows per core
TAPS = [(ky, kx) for ky in (-1, 0, 1) for kx in (-1, 0, 1)]
NPC = 4             # fixup/add/scan pieces
PCR = OH // NPC     # rows per piece


def _r12(x):
    """Round fp32 to float32r precision (11 explicit mantissa bits, RNE)."""
    u = np.ascontiguousarray(x, dtype=np.float32).view(np.uint32)
    r = (u + 0x7FF + ((u >> 12) & 1)) & 0xFFFFF000
    return r.view(np.float32)


def _fix_multiwaits(nc):
    """walrus in this container accepts at most ONE sem wait per
    instruction; split extras onto same-engine nops placed just before."""

    def steal_nop(eng):
        bi = nc.engines[eng].nop()
        ins = bi.ins
        cur = nc.cur_bb.bb
        lst = cur.instructions
        assert lst[-1] is ins or lst[-1].name == ins.name
        cur.instructions = lst[:-1]
        return ins

    for fn in nc.m.functions:
        for bb in fn.blocks:
            out = []
            changed = False
            for inst in bb.instructions:
                si = inst.sync_info
                waits = list(si.on_wait) if si is not None and si.on_wait else []
                if len(waits) > 1:
                    for wv in waits[:-1]:
                        nop = steal_nop(inst.engine)
                        nop.sync_info = bass_rust.SyncInfo(on_wait=[wv], on_update=[])
                        out.append(nop)
                    inst.sync_info = bass_rust.SyncInfo(
                        on_wait=[waits[-1]], on_update=list(si.on_update or [])
                    )
                    changed = True
                out.append(inst)
            if changed:
                bb.instructions = out
    return nc


def _emit_conv(nc, psum, rhs_flats, rhs_sizes, terms, nt, nchunks, epilogue,
               order=None):
    """Flat-chunked conv: for each output chunk accumulate all terms'
    matmuls in one PSUM tile, then run epilogue(chunk_idx, psum_ap).

    terms: list of (rhs_idx, lhsT_ap, off) where the term reads
    rhs_flats[rhs_idx][:, q+off : q+off+nt] for out positions [q, q+nt).
    Edge terms are trimmed by one element at the slab ends.
    order: chunk iteration order (default ascending).
    """
    for ci in (range(nchunks) if order is None else order):
        q = ci * nt
        full, part = [], []
        for rhs_idx, lhsT, off in terms:
            o = q + off
            lo = max(0, -o)
            hi = min(nt, rhs_sizes[rhs_idx] - o)
            # fp32r matmul dst needs even offset+length; the extra trimmed
            # elements only ever read zero pad positions
            lo += lo & 1
            hi -= (hi - lo) & 1
            (full if (lo == 0 and hi == nt) else part).append(
                (rhs_idx, lhsT, o, lo, hi)
            )
        seq = [full[0]] + part + full[1:]
        pt = psum.tile([P, 512], F32, tag="ps")
        n = len(seq)
        for i, (rhs_idx, lhsT, o, lo, hi) in enumerate(seq):
            nc.tensor.matmul(
                pt[:, lo:hi],
                lhsT,
                rhs_flats[rhs_idx][:, o + lo : o + hi],
                start=(i == 0),
                stop=(i == n - 1),
            )
        epilogue(ci, pt[:, :nt])


def build_nc():
    nc = bass.Bass()

    xs = nc.dram_tensor("xs", [2, P, HS, WP], F32R, kind="ExternalInput")
    wp1 = nc.dram_tensor("wp1", [P, 2, 9, P], F32R, kind="ExternalInput")
    wp2 = nc.dram_tensor("wp2", [P, 2, 9, P], F32R, kind="ExternalInput")
    wc1 = nc.dram_tensor("wc1", [P, 2, 9, P], F32R, kind="ExternalInput")
    wc2 = nc.dram_tensor("wc2", [P, 2, 2, P], F32R, kind="ExternalInput")
    wp3 = nc.dram_tensor("wp3", [P, 2, 2, 9, P], F32R, kind="ExternalInput")
    bm = nc.dram_tensor("bm", [P, 9], F32, kind="ExternalInput")
    o = nc.dram_tensor("o", [2, P, OH, W], F32, kind="ExternalOutput")

    groups = [[0, 1], [2, 3], [4, 5], [6, 7]]

    with TileContext(nc) as tc:
        with (
            tc.tile_pool(name="const", bufs=1) as cpool,
            tc.tile_pool(name="wt", bufs=2) as wpool,
            tc.tile_pool(name="psum", bufs=6, space="PSUM") as psum,
            tc.tile_pool(name="scratch", bufs=1) as spool_s,
            tc.tile_pool(name="dram", bufs=1, space="DRAM") as dpool,
            tc.tile_pool(name="sbig", bufs=1) as spool,
            tc.tile_pool(name="x", bufs=1) as xpool,
        ):
            # ---- constants: one tiny DMA ----
            bmt = cpool.tile([P, 9], F32)
            nc.sync.dma_start(bmt[:], bm[:])
            bt = bmt[:, 0:6]
            m_top = bmt[:, 6:7]
            m_bot = bmt[:, 7:8]
            coloff = bmt[:, 8:9]
            wc2t = cpool.tile([P, 2, 2, P], F32R)

            # ---- weights: slot order wp1,wp2 (so wc1/wp3a later reuse the
            # slot freed at p1/p2 end) but wp2's DMA issues first — the two
            # early p2 chunks need it before conv p1 starts ----
            wp1t = wpool.tile([P, 2, 9, P], F32R, tag="w")
            wp2t = wpool.tile([P, 2, 9, P], F32R, tag="w")
            nc.sync.dma_start(wp2t[:], wp2[:])

            # ---- PE pre-warm: dummy matmuls on wp2 release the HAM clock
            # gate and bridge seamlessly into the first real chunk ----
            wp2fl = wp2t[:].rearrange("p a b c -> p (a b c)")
            ptw = psum.tile([P, 512], F32, tag="ps")
            for _ in range(3):
                nc.tensor.matmul(
                    ptw[:, 0:512], wp2fl[:, 0:P], wp2fl[:, 0:512],
                    start=True, stop=True,
                )

            def wtile(src):
                t = wpool.tile([P, 2, 9, P], F32R, tag="w")
                nc.sync.dma_start(t[:], src[:])
                return t

            # ---- x slabs, highest rows first (conv p1 runs reverse); the
            # rows-0 piece comes third so p2's chunk-0 (exchange rows 0,1)
            # can run early too ----
            xt = [xpool.tile([P, HS, WP], F32R, name=f"x{s}", tag=f"x{s}") for s in range(2)]
            piece_order = [HS - 6, HS - 12, 0] + list(range(HS - 18, 0, -6))
            for k, r0 in enumerate(piece_order):
                for s in range(2):
                    nc.sync.dma_start(xt[s][:, r0:r0 + 6, :], xs[s, :, r0:r0 + 6, :])
                if k == 0:
                    nc.sync.dma_start(wp1t[:], wp1[:])
                elif k == 2:
                    nc.sync.dma_start(wc2t[:], wc2[:])
            xf = [t[:].rearrange("p h w -> p (h w)") for t in xt]
            xsz = HS * WP

            # ---- s slab (conv p1 writes rows 2..65; becomes cp1, then s) ----
            st = spool.tile([P, SS, WP], F32R)
            sf = st[:].rearrange("p h w -> p (h w)")

            NT_A, NCH_A = 416, (OH * WP) // 416

            with tc.tile_pool(name="p2", bufs=1) as p2pool:
                p2t = p2pool.tile([P, OH, WP], F32R)
                p2f = p2t[:].rearrange("p h w -> p (h w)")
                # scan reset mask: 1 everywhere, 0 at the two pad cols of
                # each row (values are post-relu >= 0 so a 0-reset == -inf)
                mk = p2pool.tile([P, PCR, WP], F32)
                nc.vector.memset(mk[:], 1.0)
                nc.vector.tensor_scalar_mul(mk[:, :, 0:1], mk[:, :, 0:1], 0.0)
                nc.vector.tensor_scalar_mul(
                    mk[:, :, WP - 1 : WP], mk[:, :, WP - 1 : WP], 0.0
                )
                mkf = mk[:].rearrange("p h w -> p (h w)")

                def conv_branch(wt_tile, bias_col, out_flat, out_base, order=None):
                    terms = []
                    for s in range(2):
                        for t_i, (ky, kx) in enumerate(TAPS):
                            terms.append(
                                (s, wt_tile[:, s, t_i, :], (1 + ky) * WP + kx)
                            )

                    def epi(ci, pap):
                        ncols = pap.shape[-1]
                        nc.scalar.activation(
                            out_flat[:, out_base + ci * NT_A : out_base + ci * NT_A + ncols],
                            pap,
                            mybir.ActivationFunctionType.Relu,
                            bias=bias_col,
                        )

                    _emit_conv(nc, psum, xf, [xsz, xsz], terms, NT_A, NCH_A, epi,
                               order=order)

                # p2's exchange chunks (rows 62,63 and 0,1) run before conv
                # p1 so the collective can fire the moment the chain ends
                conv_branch(wp2t, bt[:, 1:2], p2f, 0, order=[NCH_A - 1, 0])
                # W-cummax of the 4 exchange rows (in place, reversed)
                for h in (OH - 2, OH - 1, 0, 1):
                    row = p2t[:, h, 1 : WP - 1]
                    nc.vector.tensor_tensor_scan(
                        row[:, ::-1],
                        row[:, ::-1],
                        row[:, ::-1],
                        -1e30,
                        mybir.AluOpType.max,
                        mybir.AluOpType.bypass,
                    )

                # conv p1 -> s rows 2..65, reverse chunk order
                conv_branch(wp1t, bt[:, 0:1], sf, 2 * WP,
                            order=range(NCH_A - 1, -1, -1))
                # reverse cummax over H (rows 64 -> 2), in place; chain op h
                # only needs the conv chunks covering rows h,h+1, so under
                # reverse chunk order this interleaves with conv p1 on DVE
                for h in range(OH, 1, -1):
                    nc.vector.tensor_tensor(
                        st[:, h, :],
                        st[:, h, :],
                        st[:, h + 1, :],
                        mybir.AluOpType.max,
                    )

                # ---- pairwise exchange ----
                # C slots: 0,1 cp1local first2 rows (*mb); 2,3 cp1local last2
                # (*mt); 4,5 cp2 first2 (*mb); 6,7 cp2 last2 (*mt).
                # bf16 payload: halves the collective time; only halo rows
                # and the colmax are affected (~0.4% rounding, tol is 2e-2).
                BF16 = mybir.dt.bfloat16
                ct = spool_s.tile([P, 8, WP], BF16, tag="exch")
                for k, (src, m) in enumerate(
                    [
                        (st[:, 2, :], m_bot),
                        (st[:, 3, :], m_bot),
                        (st[:, 2 + OH - 2, :], m_top),
                        (st[:, 2 + OH - 1, :], m_top),
                        (p2t[:, 0, :], m_bot),
                        (p2t[:, 1, :], m_bot),
                        (p2t[:, OH - 2, :], m_top),
                        (p2t[:, OH - 1, :], m_top),
                    ]
                ):
                    nc.vector.tensor_scalar_mul(ct[:, k, :], src, m)
                cc_in = dpool.tile([P, 8, WP], BF16)
                cc_out = dpool.tile([P, 8, WP], BF16)
                nc.sync.dma_start(cc_in[:], ct[:])
                nc.gpsimd.collective_compute(
                    "AllReduce",
                    mybir.AluOpType.add,
                    replica_groups=groups,
                    ins=[cc_in[:]],
                    outs=[cc_out[:]],
                )
                rt = spool_s.tile([P, 8, WP], BF16, tag="exch")
                nc.sync.dma_start(rt[:], cc_out[:])

                # u = R[0] + coloff (top cores: partner colmax; bottom: -inf)
                u = spool_s.tile([P, WP], F32R, tag="u")
                nc.vector.tensor_scalar_add(u[:], rt[:, 0, :], coloff)

                # conv p2's remaining chunks run while the collective is in
                # flight and the pieces below drain
                conv_branch(wp2t, bt[:, 1:2], p2f, 0,
                            order=range(1, NCH_A - 1))

                # per 16-row piece: bulk W-cummax (flat reversed scan with
                # mask-mult state reset at the pad cols; re-scanning the
                # exchange rows is idempotent), cp1 colmax fixup, s=cp1+cp2,
                # pad-col zeroing — piece 0 first so conv c1's first chunks
                # are unblocked before conv p2 even finishes.
                cm = spool_s.tile([P, WP], F32R, tag="cm")
                h0 = spool_s.tile([P, WP], F32R, tag="h0")
                h1 = spool_s.tile([P, WP], F32R, tag="h1")
                for pc in range(NPC):
                    seg = p2f[:, pc * PCR * WP : (pc + 1) * PCR * WP]
                    nc.vector.tensor_tensor_scan(
                        seg[:, ::-1],
                        seg[:, ::-1],
                        mkf[:, ::-1],
                        0.0,
                        mybir.AluOpType.max,
                        mybir.AluOpType.mult,
                    )
                    r0 = 2 + pc * PCR
                    sp = st[:, r0 : r0 + PCR, :]
                    nc.vector.tensor_tensor(
                        sp,
                        sp,
                        u[:, None, :].to_broadcast((P, PCR, WP)),
                        mybir.AluOpType.max,
                    )
                    if pc == 0:
                        # own colmax (= fixed cp1 row 0), snapshotted before
                        # the add overwrites st row 2
                        nc.vector.tensor_copy(cm[:], st[:, 2, :])
                    nc.vector.tensor_tensor(
                        sp,
                        sp,
                        p2t[:, pc * PCR : pc * PCR + PCR, :],
                        mybir.AluOpType.add,
                    )
                    nc.vector.tensor_scalar_mul(
                        st[:, r0 : r0 + PCR, 0:1], st[:, r0 : r0 + PCR, 0:1], 0.0
                    )
                    nc.vector.tensor_scalar_mul(
                        st[:, r0 : r0 + PCR, WP - 1 : WP],
                        st[:, r0 : r0 + PCR, WP - 1 : WP],
                        0.0,
                    )
                    if pc == 0:
                        # halo rows right after piece 0 (conv c1 chunk 0
                        # needs them).  above halo (bottom cores):
                        # max(partner cp1local last2, own colmax) + partner
                        # cp2 last2, *mb
                        for j, dst_row in ((0, 0), (1, 1)):
                            nc.vector.tensor_tensor(
                                h0[:], rt[:, 2 + j, :], cm[:], mybir.AluOpType.max
                            )
                            nc.vector.tensor_tensor(
                                h0[:], h0[:], rt[:, 6 + j, :], mybir.AluOpType.add
                            )
                            nc.vector.tensor_scalar_mul(
                                st[:, dst_row, :], h0[:], m_bot
                            )
                        # below halo (top cores): partner cp1local first2 +
                        # cp2 first2
                        for j, dst_row in ((0, SS - 2), (1, SS - 1)):
                            nc.vector.tensor_tensor(
                                h1[:], rt[:, 0 + j, :], rt[:, 4 + j, :],
                                mybir.AluOpType.add,
                            )
                            nc.vector.tensor_scalar_mul(
                                st[:, dst_row, :], h1[:], m_top
                            )
                        # zero pad cols of the 4 halo rows (in-place *0:
                        # memset may not produce fp32r-matmul operands under
                        # this walrus)
                        for rr in (0, SS - 2):
                            nc.vector.tensor_scalar_mul(
                                st[:, rr : rr + 2, 0:1],
                                st[:, rr : rr + 2, 0:1],
                                0.0,
                            )
                            nc.vector.tensor_scalar_mul(
                                st[:, rr : rr + 2, WP - 1 : WP],
                                st[:, rr : rr + 2, WP - 1 : WP],
                                0.0,
                            )

            # ---- conv c1 + c2 -> r_ext ----
            wc1t = wtile(wc1)
            NT_C, NCH_C = 390, ((OH + 2) * WP) // 390
            ssz = SS * WP
            with tc.tile_pool(name="r", bufs=1) as rpool:
                rt2 = [rpool.tile([P, HS, WP], F32R, name=f"r{i}", tag=f"r{i}") for i in range(2)]
                rf = [t[:].rearrange("p h w -> p (h w)") for t in rt2]
                for oh_half in range(2):
                    terms = []
                    for t_i, (ky, kx) in enumerate(TAPS):
                        terms.append((0, wc1t[:, oh_half, t_i, :], (1 + ky) * WP + kx))
                    for s in range(2):
                        terms.append((1 + s, wc2t[:, s, oh_half, :], 0))

                    def epi(ci, pap, oh_half=oh_half):
                        ncols = pap.shape[-1]
                        nc.scalar.activation(
                            rf[oh_half][:, ci * NT_C : ci * NT_C + ncols],
                            pap,
                            mybir.ActivationFunctionType.Relu,
                            bias=bt[:, 2 + oh_half : 3 + oh_half],
                        )

                    _emit_conv(
                        nc, psum, [sf, xf[0], xf[1]], [ssz, xsz, xsz],
                        terms, NT_C, NCH_C, epi,
                    )
                    # mask invalid halo rows, zero pad cols (per half so the
                    # ops overlap the other half's matmuls)
                    t = rt2[oh_half]
                    nc.vector.tensor_scalar_mul(t[:, 0, :], t[:, 0, :], m_bot)
                    nc.vector.tensor_scalar_mul(
                        t[:, HS - 1, :], t[:, HS - 1, :], m_top
                    )
                    nc.vector.tensor_scalar_mul(t[:, :, 0:1], t[:, :, 0:1], 0.0)
                    nc.vector.tensor_scalar_mul(
                        t[:, :, WP - 1 : WP], t[:, :, WP - 1 : WP], 0.0
                    )

                # ---- conv p3 -> per-half staged rows -> out ----
                # x is no longer needed; its SBUF is reused for the two
                # output stages.  NT 416 (vs 260) cuts matmul count 1152->720
                # and the 8-row output DMAs cut issue count 64->16.
                wp3a = wpool.tile([P, 2, 9, P], F32R, tag="w")
                nc.sync.dma_start(wp3a[:], wp3[:, 0])
                wp3b = wpool.tile([P, 2, 9, P], F32R, tag="w")
                nc.sync.dma_start(wp3b[:], wp3[:, 1])
                NT_O, NCH_O = 416, (OH * WP) // 416
                rsz = HS * WP
                if True:
                    for oh_half, wtile_ in ((0, wp3a), (1, wp3b)):
                        # stage reuses the x slab's SBUF slot (same tag);
                        # the alloc waits for conv c1's last x access
                        stg = xpool.tile([P, OH, WP], F32, tag=f"x{oh_half}")
                        sgf = stg[:].rearrange("p h w -> p (h w)")
                        terms = []
                        for s in range(2):
                            for t_i, (ky, kx) in enumerate(TAPS):
                                terms.append(
                                    (s, wtile_[:, s, t_i, :], (1 + ky) * WP + kx)
                                )

                        def epi(ci, pap, sgf=sgf, oh_half=oh_half):
                            ncols = pap.shape[-1]
                            nc.scalar.activation(
                                sgf[:, ci * NT_O : ci * NT_O + ncols],
                                pap,
                                mybir.ActivationFunctionType.Relu,
                                bias=bt[:, 4 + oh_half : 5 + oh_half],
                            )

                        _emit_conv(nc, psum, rf, [rsz, rsz], terms, NT_O,
                                   NCH_O, epi)
                        for r0 in range(0, OH, 8):
                            nc.sync.dma_start(
                                o[oh_half, :, r0 : r0 + 8, :],
                                stg[:, r0 : r0 + 8, 1 : WP - 1],
                            )

    _fix_multiwaits(nc)
    return nc


_NC = None


def _get_nc():
    global _NC
    if _NC is None:
        _NC = build_nc()
    return _NC


def _fold_bn(w, g, b, m, v):
    s = (g / np.sqrt(v + EPS)).astype(np.float32)
    t = (b - m * s).astype(np.float32)
    return w * s[:, None, None, None], t


def kernel(**inputs):
    x = np.asarray(inputs["x"], np.float32)

    w_p1, t_p1 = _fold_bn(
        np.asarray(inputs["w_p1"], np.float32),
        inputs["g_p1"], inputs["b_p1"], inputs["m_p1"], inputs["v_p1"],
    )
    w_p2, t_p2 = _fold_bn(
        np.asarray(inputs["w_p2"], np.float32),
        inputs["g_p2"], inputs["b_p2"], inputs["m_p2"], inputs["v_p2"],
    )
    w_c1, t_c1 = _fold_bn(
        np.asarray(inputs["w_c1"], np.float32),
        inputs["g_c1"], inputs["b_c1"], inputs["m_c1"], inputs["v_c1"],
    )
    w_c2, t_c2 = _fold_bn(
        np.asarray(inputs["w_c2"], np.float32),
        inputs["g_c2"], inputs["b_c2"], inputs["m_c2"], inputs["v_c2"],
    )
    w_p3, t_p3 = _fold_bn(
        np.asarray(inputs["w_p3"], np.float32),
        inputs["g_p3"], inputs["b_p3"], inputs["m_p3"], inputs["v_p3"],
    )

    # weight layouts (see build_nc): contraction channel on partitions
    def lay3x3(wf, cout_half):
        # wf [O, I, 3, 3] -> [128ci, n_i_sub, (oh?), 9, 128co]
        O, I = wf.shape[0], wf.shape[1]
        a = wf.reshape(O // P, P, I // P, P, 3, 3)  # [ohs, co, s, ci, ky, kx]
        a = a.transpose(3, 0, 2, 4, 5, 1)  # [ci, ohs, s, ky, kx, co]
        return np.ascontiguousarray(a)

    wp1a = lay3x3(w_p1, 1).reshape(P, 1, 2, 9, P)[:, 0]          # [128,2,9,128]
    wp2a = lay3x3(w_p2, 1).reshape(P, 1, 2, 9, P)[:, 0]
    wc1a = lay3x3(w_c1, 2).reshape(P, 2, 1, 9, P)[:, :, 0]       # [128,2oh,9,128]
    wp3a = lay3x3(w_p3, 2)                                        # [128,2oh,2s,9,128]
    wc2a = np.ascontiguousarray(
        w_c2[:, :, 0, 0].reshape(2, P, 2, P).transpose(3, 2, 0, 1)
    )  # [128ci, 2s, 2oh, 128co]

    bias = np.zeros((P, 6), np.float32)
    bias[:, 0] = t_p1
    bias[:, 1] = t_p2
    bc = t_c1 + t_c2
    bias[:, 2] = bc[:P]
    bias[:, 3] = bc[P:]
    bias[:, 4] = t_p3[:P]
    bias[:, 5] = t_p3[P:]

    # x slabs per core, W-padded + H halo, rounded to f32r
    xr = x.reshape(B, 2, P, H, W)
    slabs = np.zeros((B, 2, 2, P, HS, WP), np.float32)  # [b, half, s, p, h, w]
    for b in range(B):
        slabs[b, 0, :, :, 1:HS, 1 : WP - 1] = xr[b, :, :, 0:65, :]
        slabs[b, 1, :, :, 0 : HS - 1, 1 : WP - 1] = xr[b, :, :, 63:128, :]
    slabs = _r12(slabs)

    wmaps = {
        "wp1": _r12(wp1a),
        "wp2": _r12(wp2a),
        "wc1": _r12(wc1a),
        "wc2": _r12(wc2a),
        "wp3": _r12(wp3a),
    }
    in_maps = []
    for b in range(B):
        for half in range(2):
            bmv = np.zeros((P, 9), np.float32)
            bmv[:, 0:6] = bias
            if half == 0:  # top
                bmv[:, 6] = 1.0  # m_top
                bmv[:, 8] = 0.0
            else:  # bottom
                bmv[:, 7] = 1.0  # m_bot
                bmv[:, 8] = -1e30
            in_maps.append({"xs": slabs[b, half], "bm": bmv, **wmaps})

    global _last_in_maps
    _last_in_maps = in_maps

    nc = _get_nc()
    res = run_bass_kernel_spmd(nc, in_maps, list(range(8)))

    out = np.empty((B, CIN, H, W), np.float32)
    for b in range(B):
        out[b, :, 0:OH] = res.results[2 * b]["o"].reshape(CIN, OH, W)
        out[b, :, OH:H] = res.results[2 * b + 1]["o"].reshape(CIN, OH, W)
    return out


if __name__ == "__main__":
    import reference

    inp = {k: np.asarray(v) for k, v in reference.setup_inputs().items()}
    exp = np.asarray(reference.reference(**inp))
    got = kernel(**inp)
    err = np.abs(got - exp)
    rel = err.max() / max(np.abs(exp).max(), 1e-6)
    print("abs err max:", err.max(), "rel (vs absmax):", rel)
